# revision 28
# baseline (speedup 1.0000x reference)
"""Single-head attention layer (B=4, S=2048, D=H=1024) on 8 TRN2 NeuronCores.

Sharding: core c -> batch c//2, query-half c%2 (1024 query rows per core).
K is projected in full (transposed layout) on both cores of a batch pair
from the host-provided x^T; V is projected only for the core's own half
and the halves are exchanged with one 2-core AllGather, which has ~100us
of schedule slack before attn@V needs it. Scores are computed transposed
so softmax needs no on-chip transposes and no max-subtraction
(|scores*scale| < ~3 here).

All matmuls run in bf16 with fp32 PSUM accumulation:
  Vh[s,h]    = matmul(lhsT=xq[d,s], rhs=Wv[d,h])     (+bv via DVE bcast add)
  V          = AllGather(Vh) over pairs {2b, 2b+1}
  KT[h,k]    = matmul(lhsT=Wk[d,h], rhs=xt[d,k])     (+bk via ACT bias)
  QT[h,q]    = matmul(lhsT=Wq[d,h], rhs=xq[d,q])     (+bq via ACT bias)
  ST[k,q]    = matmul(lhsT=KT[h,k], rhs=QT[h,q])
  ET[k,q]    = exp(ST * 1/sqrt(H))
  O[q,h]     = matmul(lhsT=ET[k,q], rhs=V[k,h])      (accumulate over k)
  den[q,1]   = matmul(lhsT=ET[k,q], rhs=ones[k,1])
  out        = O * (1/den)
"""

import os

import numpy as np
import ml_dtypes

B, S, D, H = 4, 2048, 1024, 1024
NCORES = 8
PT = 128            # partition tile
CH = 512            # psum free-dim chunk (fp32 bank limit)
QH = S // 2         # rows per core
ND = D // PT        # 8 d-tiles
NHT = H // PT       # 8 h-tiles
NKT = S // PT       # 16 k/s-tiles (full sequence)
NST = QH // PT      # 8 s-tiles in this core's half
NQT = QH // PT      # 8 q-tiles per core
SCALE = 1.0 / float(np.sqrt(H))

BF16 = ml_dtypes.bfloat16

_NC = None


def _build():
    import concourse.bacc as bacc
    import concourse.mybir as mybir
    from concourse.tile import TileContext

    dt = mybir.dt
    AF = mybir.ActivationFunctionType
    GROUPS = [[0, 1], [2, 3], [4, 5], [6, 7]]

    nc = bacc.Bacc(None, target_bir_lowering=False, num_devices=NCORES,
                   num_swdge_queues=4)

    xq = nc.declare_dram_parameter("xq", [D, QH], dt.bfloat16, isOutput=False)
    wq = nc.declare_dram_parameter("wq", [D, H], dt.bfloat16, isOutput=False)
    wk = nc.declare_dram_parameter("wk", [D, H], dt.bfloat16, isOutput=False)
    wv = nc.declare_dram_parameter("wv", [D, H], dt.bfloat16, isOutput=False)
    bqr = nc.declare_dram_parameter("bqr", [PT, NHT], dt.float32, isOutput=False)
    bkr = nc.declare_dram_parameter("bkr", [PT, NHT], dt.float32, isOutput=False)
    bvb = nc.declare_dram_parameter("bvb", [PT, H], dt.bfloat16, isOutput=False)
    y = nc.declare_dram_parameter("y", [QH, H], dt.float32, isOutput=True)

    with TileContext(nc) as tc:
        with (
            tc.tile_pool(name="px", bufs=ND) as px,        # xt tiles then ET tiles
            tc.tile_pool(name="pxq", bufs=ND) as pxq,
            tc.tile_pool(name="pw", bufs=3 * ND) as pw,
            tc.tile_pool(name="pqt", bufs=NHT) as pqt,
            tc.tile_pool(name="pkt", bufs=NHT) as pkt,
            tc.tile_pool(name="pv", bufs=NKT) as pv,
            tc.tile_pool(name="pmisc", bufs=1) as pmisc,
            tc.tile_pool(name="phalf", bufs=4) as phalf,
            tc.tile_pool(name="pstage", bufs=4) as pstage,
            tc.tile_pool(name="prd", bufs=2) as prd,
            tc.tile_pool(name="pdram", bufs=1, space="DRAM") as pdram,
            tc.tile_pool(name="psum", bufs=8, space="PSUM") as pp,
        ):
            # ---- DRAM bounce tensors for the K/V exchange. K is exchanged
            # in KT layout ([h, own-k-half]) so the AllGather's dim-0 concat
            # lands on the h axis: reloading needs only contiguous DMAs. ----
            kh_d = [pdram.tile([H, QH // 2], dt.bfloat16, tag=f"khd{i}",
                               name="khd") for i in range(2)]
            kf_d = [pdram.tile([2 * H, QH // 2], dt.bfloat16, tag=f"kfd{i}",
                               name="kfd") for i in range(2)]
            vh_d = pdram.tile([QH, H], dt.bfloat16, tag="vhd")
            vf_d = pdram.tile([S, H], dt.bfloat16, tag="vfd")

            # ---- loads, ordered by first use: (xq,wv) d-interleaved for
            # the V-half matmuls, biases (needed ~30us in), wk, xt, wq ----
            xq_t = []
            w_t = {}
            bq_t = bk_t = bv_t = ones_t = None
            for d in range(ND):
                t = pxq.tile([PT, QH], dt.bfloat16, tag="xq", name="xqt")
                nc.sync.dma_start(out=t[:], in_=xq[d * PT:(d + 1) * PT, :])
                xq_t.append(t)
                tw = pw.tile([PT, H], dt.bfloat16, tag="w", name="wt")
                nc.sync.dma_start(out=tw[:], in_=wk[d * PT:(d + 1) * PT, :])
                w_t["wk", d] = tw
                if d == 0:
                    bv_t = pmisc.tile([PT, H], dt.bfloat16, tag="bv")
                    nc.sync.dma_start(out=bv_t[:], in_=bvb[:, :])
                    bk_t = pmisc.tile([PT, NHT], dt.float32, tag="bk")
                    nc.sync.dma_start(out=bk_t[:], in_=bkr[:, :])
                    bq_t = pmisc.tile([PT, NHT], dt.float32, tag="bq")
                    nc.sync.dma_start(out=bq_t[:], in_=bqr[:, :])
                    ones_t = pmisc.tile([PT, CH], dt.bfloat16, tag="ones")
                    nc.vector.memset(ones_t[:], 1.0)
                    # warm the PE's HAM clock gate during the initial DMA
                    # wait so the first real matmuls run at 2.4 GHz
                    wps = pp.tile([PT, CH], dt.float32, tag="big", name="wps")
                    for _ in range(16):
                        nc.tensor.matmul(
                            wps[0:1, :], ones_t[:, 0:1], ones_t[:, :],
                            start=True, stop=True,
                        )
            for name, hnd in (("wv", wv), ("wq", wq)):
                for d in range(ND):
                    t = pw.tile([PT, H], dt.bfloat16, tag="w", name="wt")
                    nc.sync.dma_start(out=t[:], in_=hnd[d * PT:(d + 1) * PT, :])
                    w_t[name, d] = t

            # ---- phase A1: KT-half projection, k-chunk-major with h inner
            # so the first AllGather (all h, own-k columns 0:512) can start
            # ~25us in; gathered per chunk. ----
            for c in range(2):
                for h in range(NHT):
                    ps1 = pp.tile([PT, CH], dt.float32, tag="big", name="psb")
                    for d in range(ND):
                        lhs = w_t["wk", d][:, h * PT:(h + 1) * PT]
                        nc.tensor.matmul(
                            ps1[:], lhs, xq_t[d][:, c * CH:(c + 1) * CH],
                            start=(d == 0), stop=(d == ND - 1),
                        )
                    with tc.high_priority():
                        halfc = phalf.tile([PT, CH], dt.bfloat16, tag="half",
                                           name="halfc")
                        nc.scalar.activation(
                            halfc[:], ps1[:], AF.Identity,
                            bias=bk_t[:, h:h + 1],
                        )
                        nc.gpsimd.dma_start(
                            out=kh_d[c][h * PT:(h + 1) * PT, :], in_=halfc[:],
                        )
                with tc.high_priority():
                    nc.gpsimd.collective_compute(
                        "AllGather", mybir.AluOpType.bypass,
                        replica_groups=GROUPS,
                        ins=[kh_d[c][:]], outs=[kf_d[c][:]],
                    )

            # ---- phase A2: V-half projection (d-major, two 4-s-tile blocks
            # so only 8 PSUM groups are live), export + single AllGather ----
            for vb in range(2):
                sts = range(vb * NST // 2, (vb + 1) * NST // 2)
                ps = {(st, hc): pp.tile([PT, CH], dt.float32, tag="big", name="psb")
                      for st in sts for hc in range(2)}
                for d in range(ND):
                    for st in sts:
                        lhs = xq_t[d][:, st * PT:(st + 1) * PT]
                        for hc in range(2):
                            nc.tensor.matmul(
                                ps[st, hc][:], lhs,
                                w_t["wv", d][:, hc * CH:(hc + 1) * CH],
                                start=(d == 0), stop=(d == ND - 1),
                            )
                with tc.high_priority():
                    for st in sts:
                        half = phalf.tile([PT, H], dt.bfloat16, tag="halfv",
                                          name="halfv")
                        for hc in range(2):
                            nc.vector.tensor_add(
                                half[:, hc * CH:(hc + 1) * CH], ps[st, hc][:],
                                bv_t[:, hc * CH:(hc + 1) * CH],
                            )
                        nc.gpsimd.dma_start(
                            out=vh_d[st * PT:(st + 1) * PT, :], in_=half[:],
                        )
            with tc.high_priority():
                nc.gpsimd.collective_compute(
                    "AllGather", mybir.AluOpType.bypass, replica_groups=GROUPS,
                    ins=[vh_d[:]], outs=[vf_d[:]],
                )

            # ---- phase A3: Q^T projection ----
            qt_t = []
            for h in range(NHT):
                qtile = pqt.tile([PT, QH], dt.bfloat16, tag="qt")
                qt_t.append(qtile)
                ps = [pp.tile([PT, CH], dt.float32, tag="big", name="psb")
                      for _ in range(2)]
                for d in range(ND):
                    lhs = w_t["wq", d][:, h * PT:(h + 1) * PT]
                    for c in range(2):
                        nc.tensor.matmul(
                            ps[c][:], lhs, xq_t[d][:, c * CH:(c + 1) * CH],
                            start=(d == 0), stop=(d == ND - 1),
                        )
                for c in range(2):
                    nc.scalar.activation(
                        qtile[:, c * CH:(c + 1) * CH], ps[c][:],
                        AF.Identity, bias=bq_t[:, h:h + 1],
                    )

            # ---- KT reloads from the gathered buffer: rank r's block is
            # rows [r*H, (r+1)*H) of kf_d and holds global k in
            # [r*QH, (r+1)*QH). Rank-0 half first: B's k-tiles 0-7 need
            # only it. ----
            kt_t = [pkt.tile([PT, S], dt.bfloat16, tag="kt", name="ktile")
                    for _ in range(NHT)]
            for c in range(2):
                for r in range(2):
                    for h in range(NHT):
                        nc.sync.dma_start(
                            out=kt_t[h][:, r * QH + c * CH:
                                        r * QH + (c + 1) * CH],
                            in_=kf_d[c][r * H + h * PT:r * H + (h + 1) * PT, :],
                        )

            # ---- phase B: scores^T + exp ----
            # ET stored as 8 tiles [PT, 2*QH] (two k-tiles each), reusing
            # the xt pool slots (tag "xt").
            et_t = []
            for i in range(ND):
                et_t.append(px.tile([PT, 2 * QH], dt.bfloat16, tag="xt", name="et"))

            def et_slice(kt, q0, qn):
                return et_t[kt // 2][:, (kt % 2) * QH + q0:(kt % 2) * QH + q0 + qn]

            KT_ORDER = [0, 1, 2, 3, 8, 9, 10, 11, 4, 5, 6, 7, 12, 13, 14, 15]
            for kt in KT_ORDER:
                ps = [pp.tile([PT, CH], dt.float32, tag="big", name="psb")
                      for _ in range(2)]
                for h in range(NHT):
                    lhs = kt_t[h][:, kt * PT:(kt + 1) * PT]
                    for qc in range(2):
                        nc.tensor.matmul(
                            ps[qc][:], lhs, qt_t[h][:, qc * CH:(qc + 1) * CH],
                            start=(h == 0), stop=(h == NHT - 1),
                        )
                for qc in range(2):
                    nc.scalar.activation(
                        et_slice(kt, qc * CH, CH), ps[qc][:], AF.Exp, scale=SCALE,
                    )

            # ---- V full loads (program-after B so B's waits exclude them) ----
            v_t = []
            for st in range(NKT):
                vtile = pv.tile([PT, H], dt.bfloat16, tag="v")
                v_t.append(vtile)
                nc.sync.dma_start(
                    out=vtile[:], in_=vf_d[st * PT:(st + 1) * PT, :],
                )

            # ---- phase C: attn @ V, denominator, normalize ----
            for qt in range(NQT):
                dn = pp.tile([PT, 1], dt.float32, tag="big", name="dn")
                po = [pp.tile([PT, CH], dt.float32, tag="big", name="psb")
                      for _ in range(2)]
                for kt in range(NKT):
                    lhs = et_slice(kt, qt * PT, PT)
                    for hc in range(2):
                        nc.tensor.matmul(
                            po[hc][:], lhs, v_t[kt][:, hc * CH:(hc + 1) * CH],
                            start=(kt == 0), stop=(kt == NKT - 1),
                        )
                    nc.tensor.matmul(
                        dn[:], lhs, ones_t[:, 0:1],
                        start=(kt == 0), stop=(kt == NKT - 1),
                    )
                rd = prd.tile([PT, 1], dt.float32, tag="rd")
                nc.vector.reciprocal(rd[:], dn[:])
                for hc in range(2):
                    stage = pstage.tile([PT, CH], dt.float32, tag="st", name="stage")
                    nc.vector.tensor_scalar_mul(stage[:], po[hc][:], rd[:])
                    nc.sync.dma_start(
                        out=y[qt * PT:(qt + 1) * PT, hc * CH:(hc + 1) * CH],
                        in_=stage[:],
                    )

    return nc


def _get_nc():
    global _NC
    if _NC is None:
        nc = _build()
        nc.finalize()
        _NC = nc
    return _NC


def kernel(x, Wq, bq, Wk, bk, Wv, bv):
    from concourse.bass_utils import run_bass_kernel_spmd

    nc = _get_nc()

    wq_b = np.ascontiguousarray(Wq.astype(BF16))
    wk_b = np.ascontiguousarray(Wk.astype(BF16))
    wv_b = np.ascontiguousarray(Wv.astype(BF16))
    bq_r = np.ascontiguousarray(bq.reshape(NHT, PT).T.astype(np.float32))
    bk_r = np.ascontiguousarray(bk.reshape(NHT, PT).T.astype(np.float32))
    bv_b = np.ascontiguousarray(np.broadcast_to(bv.astype(BF16), (PT, H)))

    in_maps = []
    for c in range(NCORES):
        b, qh = divmod(c, 2)
        xq_c = np.ascontiguousarray(
            x[b, qh * QH:(qh + 1) * QH, :].T.astype(BF16))
        in_maps.append({
            "xq": xq_c,
            "wq": wq_b, "wk": wk_b, "wv": wv_b,
            "bqr": bq_r, "bkr": bk_r, "bvb": bv_b,
        })

    trace = bool(os.environ.get("BASS_KERNEL_TRACE"))
    kwargs = {}
    if trace:
        _register_ntff_hook()
        kwargs = {"trace": True, "tmpdir": os.environ.get("BASS_KERNEL_TRACE_DIR")}

    res = run_bass_kernel_spmd(nc, in_maps, list(range(NCORES)), **kwargs)
    if trace:
        kernel.last_exec_time_ns = res.exec_time_ns
        kernel.last_results = res

    out = np.empty((B, S, H), np.float32)
    for c in range(NCORES):
        b, qh = divmod(c, 2)
        out[b, qh * QH:(qh + 1) * QH, :] = res.results[c]["y"]
    return out


def _register_ntff_hook():
    """The container's antenv lacks axon_hooks; register it so trace=True
    can capture NTFF profiles through the axon PJRT library."""
    import sys
    import types

    if "antenv.axon_hooks" in sys.modules:
        return
    mod = types.ModuleType("antenv.axon_hooks")
    holder = [None]
    mod.set_axon_ntff_profile_hook = lambda h: holder.__setitem__(0, h)
    mod.get_axon_ntff_profile_hook = lambda: holder[0]
    sys.modules["antenv.axon_hooks"] = mod
    import antenv

    antenv.axon_hooks = mod
    from trn_agent_boot.trn_boot import _ntff_profile_via_ctypes

    mod.set_axon_ntff_profile_hook(_ntff_profile_via_ctypes("/opt/axon/libaxon_pjrt.so"))


# revision 29
# speedup vs baseline: 1.0080x; 1.0080x over previous
"""Single-head attention layer (B=4, S=2048, D=H=1024) on 8 TRN2 NeuronCores.

Sharding: core c -> batch c//2, query-half c%2 (1024 query rows per core).
K is projected in full (transposed layout) on both cores of a batch pair
from the host-provided x^T; V is projected only for the core's own half
and the halves are exchanged with one 2-core AllGather, which has ~100us
of schedule slack before attn@V needs it. Scores are computed transposed
so softmax needs no on-chip transposes and no max-subtraction
(|scores*scale| < ~3 here).

All matmuls run in bf16 with fp32 PSUM accumulation:
  Vh[s,h]    = matmul(lhsT=xq[d,s], rhs=Wv[d,h])     (+bv via DVE bcast add)
  V          = AllGather(Vh) over pairs {2b, 2b+1}
  KT[h,k]    = matmul(lhsT=Wk[d,h], rhs=xt[d,k])     (+bk via ACT bias)
  QT[h,q]    = matmul(lhsT=Wq[d,h], rhs=xq[d,q])     (+bq via ACT bias)
  ST[k,q]    = matmul(lhsT=KT[h,k], rhs=QT[h,q])
  ET[k,q]    = exp(ST * 1/sqrt(H))
  O[q,h]     = matmul(lhsT=ET[k,q], rhs=V[k,h])      (accumulate over k)
  den[q,1]   = matmul(lhsT=ET[k,q], rhs=ones[k,1])
  out        = O * (1/den)
"""

import os

import numpy as np
import ml_dtypes

B, S, D, H = 4, 2048, 1024, 1024
NCORES = 8
PT = 128            # partition tile
CH = 512            # psum free-dim chunk (fp32 bank limit)
QH = S // 2         # rows per core
ND = D // PT        # 8 d-tiles
NHT = H // PT       # 8 h-tiles
NKT = S // PT       # 16 k/s-tiles (full sequence)
NST = QH // PT      # 8 s-tiles in this core's half
NQT = QH // PT      # 8 q-tiles per core
SCALE = 1.0 / float(np.sqrt(H))

BF16 = ml_dtypes.bfloat16

_NC = None


def _build():
    import concourse.bacc as bacc
    import concourse.mybir as mybir
    from concourse.tile import TileContext

    dt = mybir.dt
    AF = mybir.ActivationFunctionType
    GROUPS = [[0, 1], [2, 3], [4, 5], [6, 7]]

    nc = bacc.Bacc(None, target_bir_lowering=False, num_devices=NCORES,
                   num_swdge_queues=4)

    xq = nc.declare_dram_parameter("xq", [D, QH], dt.bfloat16, isOutput=False)
    wq = nc.declare_dram_parameter("wq", [D, H], dt.bfloat16, isOutput=False)
    wk = nc.declare_dram_parameter("wk", [D, H], dt.bfloat16, isOutput=False)
    wv = nc.declare_dram_parameter("wv", [D, H], dt.bfloat16, isOutput=False)
    bqr = nc.declare_dram_parameter("bqr", [PT, NHT], dt.float32, isOutput=False)
    bkr = nc.declare_dram_parameter("bkr", [PT, NHT], dt.float32, isOutput=False)
    bvb = nc.declare_dram_parameter("bvb", [PT, H], dt.bfloat16, isOutput=False)
    y = nc.declare_dram_parameter("y", [QH, H], dt.float32, isOutput=True)

    with TileContext(nc) as tc:
        with (
            tc.tile_pool(name="px", bufs=ND) as px,        # xt tiles then ET tiles
            tc.tile_pool(name="pxq", bufs=ND) as pxq,
            tc.tile_pool(name="pw", bufs=3 * ND) as pw,
            tc.tile_pool(name="pqt", bufs=NHT) as pqt,
            tc.tile_pool(name="pkt", bufs=NHT) as pkt,
            tc.tile_pool(name="pv", bufs=NKT) as pv,
            tc.tile_pool(name="pmisc", bufs=1) as pmisc,
            tc.tile_pool(name="phalf", bufs=4) as phalf,
            tc.tile_pool(name="pstage", bufs=4) as pstage,
            tc.tile_pool(name="prd", bufs=2) as prd,
            tc.tile_pool(name="pdram", bufs=1, space="DRAM") as pdram,
            tc.tile_pool(name="psum", bufs=8, space="PSUM") as pp,
        ):
            # ---- DRAM bounce tensors for the K/V exchange. K is exchanged
            # in KT layout ([h, own-k-half]) so the AllGather's dim-0 concat
            # lands on the h axis: reloading needs only contiguous DMAs. ----
            kh_d = [pdram.tile([H, QH // 2], dt.bfloat16, tag=f"khd{i}",
                               name="khd") for i in range(2)]
            kf_d = [pdram.tile([2 * H, QH // 2], dt.bfloat16, tag=f"kfd{i}",
                               name="kfd") for i in range(2)]
            vh_d = pdram.tile([QH, H], dt.bfloat16, tag="vhd")
            vf_d = pdram.tile([S, H], dt.bfloat16, tag="vfd")

            # ---- loads, ordered by first use: (xq,wv) d-interleaved for
            # the V-half matmuls, biases (needed ~30us in), wk, xt, wq ----
            xq_t = []
            w_t = {}
            bq_t = bk_t = bv_t = ones_t = None
            for d in range(ND):
                t = pxq.tile([PT, QH], dt.bfloat16, tag="xq", name="xqt")
                nc.sync.dma_start(out=t[:], in_=xq[d * PT:(d + 1) * PT, :])
                xq_t.append(t)
                tw = pw.tile([PT, H], dt.bfloat16, tag="w", name="wt")
                nc.sync.dma_start(out=tw[:], in_=wk[d * PT:(d + 1) * PT, :])
                w_t["wk", d] = tw
                if d == 0:
                    bv_t = pmisc.tile([PT, H], dt.bfloat16, tag="bv")
                    nc.sync.dma_start(out=bv_t[:], in_=bvb[:, :])
                    bk_t = pmisc.tile([PT, NHT], dt.float32, tag="bk")
                    nc.sync.dma_start(out=bk_t[:], in_=bkr[:, :])
                    bq_t = pmisc.tile([PT, NHT], dt.float32, tag="bq")
                    nc.sync.dma_start(out=bq_t[:], in_=bqr[:, :])
                    ones_t = pmisc.tile([PT, 1], dt.bfloat16, tag="ones")
                    nc.vector.memset(ones_t[:], 1.0)
            for name, hnd in (("wv", wv), ("wq", wq)):
                for d in range(ND):
                    t = pw.tile([PT, H], dt.bfloat16, tag="w", name="wt")
                    nc.sync.dma_start(out=t[:], in_=hnd[d * PT:(d + 1) * PT, :])
                    w_t[name, d] = t

            # ---- phase A1: KT-half projection, k-chunk-major with h inner
            # so the first AllGather (all h, own-k columns 0:512) can start
            # ~25us in; gathered per chunk. ----
            for c in range(2):
                for h in range(NHT):
                    ps1 = pp.tile([PT, CH], dt.float32, tag="big", name="psb")
                    for d in range(ND):
                        lhs = w_t["wk", d][:, h * PT:(h + 1) * PT]
                        nc.tensor.matmul(
                            ps1[:], lhs, xq_t[d][:, c * CH:(c + 1) * CH],
                            start=(d == 0), stop=(d == ND - 1),
                        )
                    with tc.high_priority():
                        halfc = phalf.tile([PT, CH], dt.bfloat16, tag="half",
                                           name="halfc")
                        nc.scalar.activation(
                            halfc[:], ps1[:], AF.Identity,
                            bias=bk_t[:, h:h + 1],
                        )
                        nc.gpsimd.dma_start(
                            out=kh_d[c][h * PT:(h + 1) * PT, :], in_=halfc[:],
                        )
                with tc.high_priority():
                    nc.gpsimd.collective_compute(
                        "AllGather", mybir.AluOpType.bypass,
                        replica_groups=GROUPS,
                        ins=[kh_d[c][:]], outs=[kf_d[c][:]],
                    )

            # ---- phase A2: V-half projection (d-major, two 4-s-tile blocks
            # so only 8 PSUM groups are live), export + single AllGather ----
            for vb in range(2):
                sts = range(vb * NST // 2, (vb + 1) * NST // 2)
                ps = {(st, hc): pp.tile([PT, CH], dt.float32, tag="big", name="psb")
                      for st in sts for hc in range(2)}
                for d in range(ND):
                    for st in sts:
                        lhs = xq_t[d][:, st * PT:(st + 1) * PT]
                        for hc in range(2):
                            nc.tensor.matmul(
                                ps[st, hc][:], lhs,
                                w_t["wv", d][:, hc * CH:(hc + 1) * CH],
                                start=(d == 0), stop=(d == ND - 1),
                            )
                with tc.high_priority():
                    for st in sts:
                        half = phalf.tile([PT, H], dt.bfloat16, tag="halfv",
                                          name="halfv")
                        for hc in range(2):
                            nc.vector.tensor_add(
                                half[:, hc * CH:(hc + 1) * CH], ps[st, hc][:],
                                bv_t[:, hc * CH:(hc + 1) * CH],
                            )
                        nc.gpsimd.dma_start(
                            out=vh_d[st * PT:(st + 1) * PT, :], in_=half[:],
                        )
            with tc.high_priority():
                nc.gpsimd.collective_compute(
                    "AllGather", mybir.AluOpType.bypass, replica_groups=GROUPS,
                    ins=[vh_d[:]], outs=[vf_d[:]],
                )

            # ---- phase A3: Q^T projection ----
            qt_t = []
            for h in range(NHT):
                qtile = pqt.tile([PT, QH], dt.bfloat16, tag="qt")
                qt_t.append(qtile)
                ps = [pp.tile([PT, CH], dt.float32, tag="big", name="psb")
                      for _ in range(2)]
                for d in range(ND):
                    lhs = w_t["wq", d][:, h * PT:(h + 1) * PT]
                    for c in range(2):
                        nc.tensor.matmul(
                            ps[c][:], lhs, xq_t[d][:, c * CH:(c + 1) * CH],
                            start=(d == 0), stop=(d == ND - 1),
                        )
                for c in range(2):
                    nc.scalar.activation(
                        qtile[:, c * CH:(c + 1) * CH], ps[c][:],
                        AF.Identity, bias=bq_t[:, h:h + 1],
                    )

            # ---- KT reloads from the gathered buffer: rank r's block is
            # rows [r*H, (r+1)*H) of kf_d and holds global k in
            # [r*QH, (r+1)*QH). Rank-0 half first: B's k-tiles 0-7 need
            # only it. ----
            kt_t = [pkt.tile([PT, S], dt.bfloat16, tag="kt", name="ktile")
                    for _ in range(NHT)]
            for c in range(2):
                for r in range(2):
                    for h in range(NHT):
                        nc.sync.dma_start(
                            out=kt_t[h][:, r * QH + c * CH:
                                        r * QH + (c + 1) * CH],
                            in_=kf_d[c][r * H + h * PT:r * H + (h + 1) * PT, :],
                        )

            # ---- phase B: scores^T + exp ----
            # ET stored as 8 tiles [PT, 2*QH] (two k-tiles each), reusing
            # the xt pool slots (tag "xt").
            et_t = []
            for i in range(ND):
                et_t.append(px.tile([PT, 2 * QH], dt.bfloat16, tag="xt", name="et"))

            def et_slice(kt, q0, qn):
                return et_t[kt // 2][:, (kt % 2) * QH + q0:(kt % 2) * QH + q0 + qn]

            KT_ORDER = [0, 1, 2, 3, 8, 9, 10, 11, 4, 5, 6, 7, 12, 13, 14, 15]
            for kt in KT_ORDER:
                ps = [pp.tile([PT, CH], dt.float32, tag="big", name="psb")
                      for _ in range(2)]
                for h in range(NHT):
                    lhs = kt_t[h][:, kt * PT:(kt + 1) * PT]
                    for qc in range(2):
                        nc.tensor.matmul(
                            ps[qc][:], lhs, qt_t[h][:, qc * CH:(qc + 1) * CH],
                            start=(h == 0), stop=(h == NHT - 1),
                        )
                for qc in range(2):
                    nc.scalar.activation(
                        et_slice(kt, qc * CH, CH), ps[qc][:], AF.Exp, scale=SCALE,
                    )

            # ---- V full loads (program-after B so B's waits exclude them) ----
            v_t = []
            for st in range(NKT):
                vtile = pv.tile([PT, H], dt.bfloat16, tag="v")
                v_t.append(vtile)
                nc.sync.dma_start(
                    out=vtile[:], in_=vf_d[st * PT:(st + 1) * PT, :],
                )

            # ---- phase C: attn @ V, denominator, normalize ----
            for qt in range(NQT):
                dn = pp.tile([PT, 1], dt.float32, tag="big", name="dn")
                po = [pp.tile([PT, CH], dt.float32, tag="big", name="psb")
                      for _ in range(2)]
                for kt in range(NKT):
                    lhs = et_slice(kt, qt * PT, PT)
                    for hc in range(2):
                        nc.tensor.matmul(
                            po[hc][:], lhs, v_t[kt][:, hc * CH:(hc + 1) * CH],
                            start=(kt == 0), stop=(kt == NKT - 1),
                        )
                    nc.tensor.matmul(
                        dn[:], lhs, ones_t[:, 0:1],
                        start=(kt == 0), stop=(kt == NKT - 1),
                    )
                rd = prd.tile([PT, 1], dt.float32, tag="rd")
                nc.vector.reciprocal(rd[:], dn[:])
                for hc in range(2):
                    stage = pstage.tile([PT, CH], dt.float32, tag="st", name="stage")
                    nc.vector.tensor_scalar_mul(stage[:], po[hc][:], rd[:])
                    nc.sync.dma_start(
                        out=y[qt * PT:(qt + 1) * PT, hc * CH:(hc + 1) * CH],
                        in_=stage[:],
                    )

    return nc


def _get_nc():
    global _NC
    if _NC is None:
        nc = _build()
        nc.finalize()
        _NC = nc
    return _NC


def kernel(x, Wq, bq, Wk, bk, Wv, bv):
    from concourse.bass_utils import run_bass_kernel_spmd

    nc = _get_nc()

    wq_b = np.ascontiguousarray(Wq.astype(BF16))
    wk_b = np.ascontiguousarray(Wk.astype(BF16))
    wv_b = np.ascontiguousarray(Wv.astype(BF16))
    bq_r = np.ascontiguousarray(bq.reshape(NHT, PT).T.astype(np.float32))
    bk_r = np.ascontiguousarray(bk.reshape(NHT, PT).T.astype(np.float32))
    bv_b = np.ascontiguousarray(np.broadcast_to(bv.astype(BF16), (PT, H)))

    in_maps = []
    for c in range(NCORES):
        b, qh = divmod(c, 2)
        xq_c = np.ascontiguousarray(
            x[b, qh * QH:(qh + 1) * QH, :].T.astype(BF16))
        in_maps.append({
            "xq": xq_c,
            "wq": wq_b, "wk": wk_b, "wv": wv_b,
            "bqr": bq_r, "bkr": bk_r, "bvb": bv_b,
        })

    trace = bool(os.environ.get("BASS_KERNEL_TRACE"))
    kwargs = {}
    if trace:
        _register_ntff_hook()
        kwargs = {"trace": True, "tmpdir": os.environ.get("BASS_KERNEL_TRACE_DIR")}

    res = run_bass_kernel_spmd(nc, in_maps, list(range(NCORES)), **kwargs)
    if trace:
        kernel.last_exec_time_ns = res.exec_time_ns
        kernel.last_results = res

    out = np.empty((B, S, H), np.float32)
    for c in range(NCORES):
        b, qh = divmod(c, 2)
        out[b, qh * QH:(qh + 1) * QH, :] = res.results[c]["y"]
    return out


def _register_ntff_hook():
    """The container's antenv lacks axon_hooks; register it so trace=True
    can capture NTFF profiles through the axon PJRT library."""
    import sys
    import types

    if "antenv.axon_hooks" in sys.modules:
        return
    mod = types.ModuleType("antenv.axon_hooks")
    holder = [None]
    mod.set_axon_ntff_profile_hook = lambda h: holder.__setitem__(0, h)
    mod.get_axon_ntff_profile_hook = lambda: holder[0]
    sys.modules["antenv.axon_hooks"] = mod
    import antenv

    antenv.axon_hooks = mod
    from trn_agent_boot.trn_boot import _ntff_profile_via_ctypes

    mod.set_axon_ntff_profile_hook(_ntff_profile_via_ctypes("/opt/axon/libaxon_pjrt.so"))


# revision 30
# speedup vs baseline: 1.0095x; 1.0015x over previous
"""Single-head attention layer (B=4, S=2048, D=H=1024) on 8 TRN2 NeuronCores.

Sharding: core c -> batch c//2, query-half c%2 (1024 query rows per core).
K is projected in full (transposed layout) on both cores of a batch pair
from the host-provided x^T; V is projected only for the core's own half
and the halves are exchanged with one 2-core AllGather, which has ~100us
of schedule slack before attn@V needs it. Scores are computed transposed
so softmax needs no on-chip transposes and no max-subtraction
(|scores*scale| < ~3 here).

All matmuls run in bf16 with fp32 PSUM accumulation:
  Vh[s,h]    = matmul(lhsT=xq[d,s], rhs=Wv[d,h])     (+bv via DVE bcast add)
  V          = AllGather(Vh) over pairs {2b, 2b+1}
  KT[h,k]    = matmul(lhsT=Wk[d,h], rhs=xt[d,k])     (+bk via ACT bias)
  QT[h,q]    = matmul(lhsT=Wq[d,h], rhs=xq[d,q])     (+bq via ACT bias)
  ST[k,q]    = matmul(lhsT=KT[h,k], rhs=QT[h,q])
  ET[k,q]    = exp(ST * 1/sqrt(H))
  O[q,h]     = matmul(lhsT=ET[k,q], rhs=V[k,h])      (accumulate over k)
  den[q,1]   = matmul(lhsT=ET[k,q], rhs=ones[k,1])
  out        = O * (1/den)
"""

import os

import numpy as np
import ml_dtypes

B, S, D, H = 4, 2048, 1024, 1024
NCORES = 8
PT = 128            # partition tile
CH = 512            # psum free-dim chunk (fp32 bank limit)
QH = S // 2         # rows per core
ND = D // PT        # 8 d-tiles
NHT = H // PT       # 8 h-tiles
NKT = S // PT       # 16 k/s-tiles (full sequence)
NST = QH // PT      # 8 s-tiles in this core's half
NQT = QH // PT      # 8 q-tiles per core
SCALE = 1.0 / float(np.sqrt(H))

BF16 = ml_dtypes.bfloat16

_NC = None


def _build():
    import concourse.bacc as bacc
    import concourse.mybir as mybir
    from concourse.tile import TileContext

    dt = mybir.dt
    AF = mybir.ActivationFunctionType
    GROUPS = [[0, 1], [2, 3], [4, 5], [6, 7]]

    nc = bacc.Bacc(None, target_bir_lowering=False, num_devices=NCORES,
                   num_swdge_queues=4)

    xq = nc.declare_dram_parameter("xq", [D, QH], dt.bfloat16, isOutput=False)
    wq = nc.declare_dram_parameter("wq", [D, H], dt.bfloat16, isOutput=False)
    wk = nc.declare_dram_parameter("wk", [D, H], dt.bfloat16, isOutput=False)
    wv = nc.declare_dram_parameter("wv", [D, H], dt.bfloat16, isOutput=False)
    bqr = nc.declare_dram_parameter("bqr", [PT, NHT], dt.float32, isOutput=False)
    bkr = nc.declare_dram_parameter("bkr", [PT, NHT], dt.float32, isOutput=False)
    bvb = nc.declare_dram_parameter("bvb", [PT, H], dt.bfloat16, isOutput=False)
    y = nc.declare_dram_parameter("y", [QH, H], dt.float32, isOutput=True)

    with TileContext(nc) as tc:
        with (
            tc.tile_pool(name="px", bufs=ND) as px,        # xt tiles then ET tiles
            tc.tile_pool(name="pxq", bufs=ND) as pxq,
            tc.tile_pool(name="pw", bufs=3 * ND) as pw,
            tc.tile_pool(name="pqt", bufs=NHT) as pqt,
            tc.tile_pool(name="pkt", bufs=NHT) as pkt,
            tc.tile_pool(name="pv", bufs=NKT) as pv,
            tc.tile_pool(name="pmisc", bufs=1) as pmisc,
            tc.tile_pool(name="phalf", bufs=4) as phalf,
            tc.tile_pool(name="pstage", bufs=4) as pstage,
            tc.tile_pool(name="prd", bufs=2) as prd,
            tc.tile_pool(name="pdram", bufs=1, space="DRAM") as pdram,
            tc.tile_pool(name="psum", bufs=8, space="PSUM") as pp,
        ):
            # ---- DRAM bounce tensors for the K/V exchange. K is exchanged
            # in KT layout ([h, own-k-half]) so the AllGather's dim-0 concat
            # lands on the h axis: reloading needs only contiguous DMAs. ----
            kh_d = [pdram.tile([H, QH // 2], dt.bfloat16, tag=f"khd{i}",
                               name="khd") for i in range(2)]
            kf_d = [pdram.tile([2 * H, QH // 2], dt.bfloat16, tag=f"kfd{i}",
                               name="kfd") for i in range(2)]
            vh_d = pdram.tile([QH, H], dt.bfloat16, tag="vhd")
            vf_d = pdram.tile([S, H], dt.bfloat16, tag="vfd")

            # ---- loads, ordered by first use: (xq,wv) d-interleaved for
            # the V-half matmuls, biases (needed ~30us in), wk, xt, wq ----
            xq_t = []
            w_t = {}
            bq_t = bk_t = bv_t = ones_t = None
            for d in range(ND):
                t = pxq.tile([PT, QH], dt.bfloat16, tag="xq", name="xqt")
                tw = pw.tile([PT, H], dt.bfloat16, tag="w", name="wt")
                if d == 0:
                    # split the first tiles in half so the very first matmul
                    # (needs xq[0][:, 0:512] and wk[0][:, 0:128]) starts
                    # ~3us earlier via subtile deps
                    nc.sync.dma_start(out=t[:, 0:QH // 2],
                                      in_=xq[0:PT, 0:QH // 2])
                    nc.sync.dma_start(out=tw[:, 0:H // 2],
                                      in_=wk[0:PT, 0:H // 2])
                    nc.sync.dma_start(out=t[:, QH // 2:QH],
                                      in_=xq[0:PT, QH // 2:QH])
                    nc.sync.dma_start(out=tw[:, H // 2:H],
                                      in_=wk[0:PT, H // 2:H])
                else:
                    nc.sync.dma_start(out=t[:],
                                      in_=xq[d * PT:(d + 1) * PT, :])
                    nc.sync.dma_start(out=tw[:],
                                      in_=wk[d * PT:(d + 1) * PT, :])
                xq_t.append(t)
                w_t["wk", d] = tw
                if d == 0:
                    bv_t = pmisc.tile([PT, H], dt.bfloat16, tag="bv")
                    nc.sync.dma_start(out=bv_t[:], in_=bvb[:, :])
                    bk_t = pmisc.tile([PT, NHT], dt.float32, tag="bk")
                    nc.sync.dma_start(out=bk_t[:], in_=bkr[:, :])
                    bq_t = pmisc.tile([PT, NHT], dt.float32, tag="bq")
                    nc.sync.dma_start(out=bq_t[:], in_=bqr[:, :])
                    ones_t = pmisc.tile([PT, 1], dt.bfloat16, tag="ones")
                    nc.vector.memset(ones_t[:], 1.0)
            for name, hnd in (("wv", wv), ("wq", wq)):
                for d in range(ND):
                    t = pw.tile([PT, H], dt.bfloat16, tag="w", name="wt")
                    nc.sync.dma_start(out=t[:], in_=hnd[d * PT:(d + 1) * PT, :])
                    w_t[name, d] = t

            # ---- phase A1: KT-half projection, k-chunk-major with h inner
            # so the first AllGather (all h, own-k columns 0:512) can start
            # ~25us in; gathered per chunk. ----
            for c in range(2):
                for h in range(NHT):
                    ps1 = pp.tile([PT, CH], dt.float32, tag="big", name="psb")
                    for d in range(ND):
                        lhs = w_t["wk", d][:, h * PT:(h + 1) * PT]
                        nc.tensor.matmul(
                            ps1[:], lhs, xq_t[d][:, c * CH:(c + 1) * CH],
                            start=(d == 0), stop=(d == ND - 1),
                        )
                    with tc.high_priority():
                        halfc = phalf.tile([PT, CH], dt.bfloat16, tag="half",
                                           name="halfc")
                        nc.scalar.activation(
                            halfc[:], ps1[:], AF.Identity,
                            bias=bk_t[:, h:h + 1],
                        )
                        nc.gpsimd.dma_start(
                            out=kh_d[c][h * PT:(h + 1) * PT, :], in_=halfc[:],
                        )
                with tc.high_priority():
                    nc.gpsimd.collective_compute(
                        "AllGather", mybir.AluOpType.bypass,
                        replica_groups=GROUPS,
                        ins=[kh_d[c][:]], outs=[kf_d[c][:]],
                    )

            # ---- phase A2: V-half projection (d-major, two 4-s-tile blocks
            # so only 8 PSUM groups are live), export + single AllGather ----
            for vb in range(2):
                sts = range(vb * NST // 2, (vb + 1) * NST // 2)
                ps = {(st, hc): pp.tile([PT, CH], dt.float32, tag="big", name="psb")
                      for st in sts for hc in range(2)}
                for d in range(ND):
                    for st in sts:
                        lhs = xq_t[d][:, st * PT:(st + 1) * PT]
                        for hc in range(2):
                            nc.tensor.matmul(
                                ps[st, hc][:], lhs,
                                w_t["wv", d][:, hc * CH:(hc + 1) * CH],
                                start=(d == 0), stop=(d == ND - 1),
                            )
                with tc.high_priority():
                    for st in sts:
                        half = phalf.tile([PT, H], dt.bfloat16, tag="halfv",
                                          name="halfv")
                        for hc in range(2):
                            nc.vector.tensor_add(
                                half[:, hc * CH:(hc + 1) * CH], ps[st, hc][:],
                                bv_t[:, hc * CH:(hc + 1) * CH],
                            )
                        nc.gpsimd.dma_start(
                            out=vh_d[st * PT:(st + 1) * PT, :], in_=half[:],
                        )
            with tc.high_priority():
                nc.gpsimd.collective_compute(
                    "AllGather", mybir.AluOpType.bypass, replica_groups=GROUPS,
                    ins=[vh_d[:]], outs=[vf_d[:]],
                )

            # ---- phase A3: Q^T projection ----
            qt_t = []
            for h in range(NHT):
                qtile = pqt.tile([PT, QH], dt.bfloat16, tag="qt")
                qt_t.append(qtile)
                ps = [pp.tile([PT, CH], dt.float32, tag="big", name="psb")
                      for _ in range(2)]
                for d in range(ND):
                    lhs = w_t["wq", d][:, h * PT:(h + 1) * PT]
                    for c in range(2):
                        nc.tensor.matmul(
                            ps[c][:], lhs, xq_t[d][:, c * CH:(c + 1) * CH],
                            start=(d == 0), stop=(d == ND - 1),
                        )
                for c in range(2):
                    nc.scalar.activation(
                        qtile[:, c * CH:(c + 1) * CH], ps[c][:],
                        AF.Identity, bias=bq_t[:, h:h + 1],
                    )

            # ---- KT reloads from the gathered buffer: rank r's block is
            # rows [r*H, (r+1)*H) of kf_d and holds global k in
            # [r*QH, (r+1)*QH). Rank-0 half first: B's k-tiles 0-7 need
            # only it. ----
            kt_t = [pkt.tile([PT, S], dt.bfloat16, tag="kt", name="ktile")
                    for _ in range(NHT)]
            for c in range(2):
                for r in range(2):
                    for h in range(NHT):
                        nc.sync.dma_start(
                            out=kt_t[h][:, r * QH + c * CH:
                                        r * QH + (c + 1) * CH],
                            in_=kf_d[c][r * H + h * PT:r * H + (h + 1) * PT, :],
                        )

            # ---- phase B: scores^T + exp ----
            # ET stored as 8 tiles [PT, 2*QH] (two k-tiles each), reusing
            # the xt pool slots (tag "xt").
            et_t = []
            for i in range(ND):
                et_t.append(px.tile([PT, 2 * QH], dt.bfloat16, tag="xt", name="et"))

            def et_slice(kt, q0, qn):
                return et_t[kt // 2][:, (kt % 2) * QH + q0:(kt % 2) * QH + q0 + qn]

            KT_ORDER = [0, 1, 2, 3, 8, 9, 10, 11, 4, 5, 6, 7, 12, 13, 14, 15]
            for kt in KT_ORDER:
                ps = [pp.tile([PT, CH], dt.float32, tag="big", name="psb")
                      for _ in range(2)]
                for h in range(NHT):
                    lhs = kt_t[h][:, kt * PT:(kt + 1) * PT]
                    for qc in range(2):
                        nc.tensor.matmul(
                            ps[qc][:], lhs, qt_t[h][:, qc * CH:(qc + 1) * CH],
                            start=(h == 0), stop=(h == NHT - 1),
                        )
                for qc in range(2):
                    nc.scalar.activation(
                        et_slice(kt, qc * CH, CH), ps[qc][:], AF.Exp, scale=SCALE,
                    )

            # ---- V full loads (program-after B so B's waits exclude them) ----
            v_t = []
            for st in range(NKT):
                vtile = pv.tile([PT, H], dt.bfloat16, tag="v")
                v_t.append(vtile)
                nc.sync.dma_start(
                    out=vtile[:], in_=vf_d[st * PT:(st + 1) * PT, :],
                )

            # ---- phase C: attn @ V, denominator, normalize ----
            for qt in range(NQT):
                dn = pp.tile([PT, 1], dt.float32, tag="big", name="dn")
                po = [pp.tile([PT, CH], dt.float32, tag="big", name="psb")
                      for _ in range(2)]
                for kt in range(NKT):
                    lhs = et_slice(kt, qt * PT, PT)
                    for hc in range(2):
                        nc.tensor.matmul(
                            po[hc][:], lhs, v_t[kt][:, hc * CH:(hc + 1) * CH],
                            start=(kt == 0), stop=(kt == NKT - 1),
                        )
                    nc.tensor.matmul(
                        dn[:], lhs, ones_t[:, 0:1],
                        start=(kt == 0), stop=(kt == NKT - 1),
                    )
                rd = prd.tile([PT, 1], dt.float32, tag="rd")
                nc.vector.reciprocal(rd[:], dn[:])
                for hc in range(2):
                    stage = pstage.tile([PT, CH], dt.float32, tag="st", name="stage")
                    nc.vector.tensor_scalar_mul(stage[:], po[hc][:], rd[:])
                    nc.sync.dma_start(
                        out=y[qt * PT:(qt + 1) * PT, hc * CH:(hc + 1) * CH],
                        in_=stage[:],
                    )

    return nc


def _get_nc():
    global _NC
    if _NC is None:
        nc = _build()
        nc.finalize()
        _NC = nc
    return _NC


def kernel(x, Wq, bq, Wk, bk, Wv, bv):
    from concourse.bass_utils import run_bass_kernel_spmd

    nc = _get_nc()

    wq_b = np.ascontiguousarray(Wq.astype(BF16))
    wk_b = np.ascontiguousarray(Wk.astype(BF16))
    wv_b = np.ascontiguousarray(Wv.astype(BF16))
    bq_r = np.ascontiguousarray(bq.reshape(NHT, PT).T.astype(np.float32))
    bk_r = np.ascontiguousarray(bk.reshape(NHT, PT).T.astype(np.float32))
    bv_b = np.ascontiguousarray(np.broadcast_to(bv.astype(BF16), (PT, H)))

    in_maps = []
    for c in range(NCORES):
        b, qh = divmod(c, 2)
        xq_c = np.ascontiguousarray(
            x[b, qh * QH:(qh + 1) * QH, :].T.astype(BF16))
        in_maps.append({
            "xq": xq_c,
            "wq": wq_b, "wk": wk_b, "wv": wv_b,
            "bqr": bq_r, "bkr": bk_r, "bvb": bv_b,
        })

    trace = bool(os.environ.get("BASS_KERNEL_TRACE"))
    kwargs = {}
    if trace:
        _register_ntff_hook()
        kwargs = {"trace": True, "tmpdir": os.environ.get("BASS_KERNEL_TRACE_DIR")}

    res = run_bass_kernel_spmd(nc, in_maps, list(range(NCORES)), **kwargs)
    if trace:
        kernel.last_exec_time_ns = res.exec_time_ns
        kernel.last_results = res

    out = np.empty((B, S, H), np.float32)
    for c in range(NCORES):
        b, qh = divmod(c, 2)
        out[b, qh * QH:(qh + 1) * QH, :] = res.results[c]["y"]
    return out


def _register_ntff_hook():
    """The container's antenv lacks axon_hooks; register it so trace=True
    can capture NTFF profiles through the axon PJRT library."""
    import sys
    import types

    if "antenv.axon_hooks" in sys.modules:
        return
    mod = types.ModuleType("antenv.axon_hooks")
    holder = [None]
    mod.set_axon_ntff_profile_hook = lambda h: holder.__setitem__(0, h)
    mod.get_axon_ntff_profile_hook = lambda: holder[0]
    sys.modules["antenv.axon_hooks"] = mod
    import antenv

    antenv.axon_hooks = mod
    from trn_agent_boot.trn_boot import _ntff_profile_via_ctypes

    mod.set_axon_ntff_profile_hook(_ntff_profile_via_ctypes("/opt/axon/libaxon_pjrt.so"))
